# revision 26
# baseline (speedup 1.0000x reference)
"""Trainium2 Bass kernel for nn_Encoder (GCN layer + MLP/BatchNorm), 8 NeuronCores.

Strategy (per core, SPMD over 8 cores):
  Phase 1 (replicated): h = x @ W_gcn written as a row-major DRAM table
    [Q*SEG, 256] bf16.  x arrives host-transposed AND pre-cast to bf16
    (xT [512, Q*SEG]), so slab loads / h-table stores are pure copies that
    ride the sync-queue hwdge, leaving the gpsimd queue free for gathers.
  Phase 2 (sharded by destination row): edges are host-bucketed by
    (source quarter, dest window of 128 rows) in QUARTER-MAJOR order.  As
    soon as quarter q's table stores land, its 98 window-gathers start
    (gpsimd dma_gather, 4 SWDGE queues round-robin, 8-deep gr ring).  A
    batched DVE op builds S^T[j,r] = val_j * (rowrel_j == r) per group; PE
    matmuls accumulate the quarter-partial into PSUM, which is transposed
    (PE) and DVE-accumulated into the column-major hat buffer.  Remaining
    phase-1 row-groups are statically interleaved into the PE/scalar
    streams during sweeps 0-2, overlapping phase 1 under the gathers.
  Phase 3: z1 = W1-matmuls (W1 stationary), BatchNorm stats reduced
    locally and AllReduce'd across the 8 cores, window recomputed and
    Relu+affine applied in one ACT op; same for W2 / BN2, final affine
    written f32 to outT [2,128,RPC].

Host side does only index/layout work: degree-balanced node->window
assignment (LPT), edge bucketing/sorting, padding, and the output unpermute.
"""

import heapq
import numpy as np
import ml_dtypes

import concourse.bacc as bacc
from concourse import bass, mybir
from concourse.bass_utils import run_bass_kernel_spmd
from concourse.library_config import mlp

BF16 = ml_dtypes.bfloat16
F32 = mybir.dt.float32
BF = mybir.dt.bfloat16
AF = mybir.ActivationFunctionType
OP = mybir.AluOpType
RACE_DETECT = True


class Cfg:
    def __init__(self, N=100000, E=3200000, SEG=25088, WPC=98, CQ=10, PH=3):
        self.N, self.E, self.SEG, self.WPC, self.CQ = N, E, SEG, WPC, CQ
        self.PH = PH
        self.CORES = 8
        self.Q = 4
        self.IN_C, self.HID, self.OUT_C = 512, 256, 256
        self.EPS = 1e-5
        self.TABROWS = self.Q * SEG          # h-table rows (>= N, %128 == 0)
        assert self.TABROWS >= N and self.TABROWS % 128 == 0
        assert SEG <= 32767 and SEG % 512 == 0  # quarter tensors batch-aligned
        self.RG = self.TABROWS // 128        # phase-1 row groups
        self.SL = (self.TABROWS + 511) // 512  # phase-1 slabs
        self.HB = (self.RG + 3) // 4         # phase-1 h-store batches of 4 rgs
        self.QSL = SEG // 512                # slabs per quarter
        self.QRG = self.RG // 4              # row groups per quarter
        assert self.SL == self.HB == 4 * self.QSL and self.RG == 4 * self.QRG
        assert self.QRG == 2 * WPC           # PE interleave covers exactly
        self.RPC = WPC * 128                 # rows per core (padded)
        self.ROWS_REAL = N // self.CORES     # real rows per core
        assert self.ROWS_REAL <= self.RPC
        self.NCH = (self.RPC + 511) // 512   # phase-3 row chunks
        self.GSLOT = 128 * CQ                # slots per (quarter, window) group
        self.NG = WPC * self.Q               # gather groups per core (q-major)
        # rings
        self.R_XT = 3                        # xts slab ring
        self.IB = 8                          # groups per idx/rv stream batch
        self.NB = (self.NG + self.IB - 1) // self.IB
        assert self.NG % self.IB == 0
        self.R_IXB = min(6, self.NB)         # idx stream batch ring
        self.R_RVB = min(8, self.NB)         # rrval stream batch ring


def _ap(t, off, pattern):
    return bass.AP(t, off, pattern)


def build(c: Cfg):
    nc = bacc.Bacc("TRN2", debug=False, num_swdge_queues=4, detect_race_conditions=RACE_DETECT)
    CQ, Q, WPC, SEG, HID, NG = c.CQ, c.Q, c.WPC, c.SEG, c.HID, c.NG

    xT = nc.declare_dram_parameter("xT", [c.IN_C, c.TABROWS], BF, isOutput=False)
    wgcn = nc.declare_dram_parameter("wgcn", [c.IN_C, HID], F32, isOutput=False)
    w1 = nc.declare_dram_parameter("w1", [HID, HID], F32, isOutput=False)
    w2 = nc.declare_dram_parameter("w2", [HID, c.OUT_C], F32, isOutput=False)
    bgcn2 = nc.declare_dram_parameter("bgcn2", [128, 2], F32, isOutput=False)
    g1v = nc.declare_dram_parameter("g1v", [128, 2], F32, isOutput=False)
    be1v = nc.declare_dram_parameter("be1v", [128, 2], F32, isOutput=False)
    g2v = nc.declare_dram_parameter("g2v", [128, 2], F32, isOutput=False)
    be2v = nc.declare_dram_parameter("be2v", [128, 2], F32, isOutput=False)
    iota_in = nc.declare_dram_parameter("iota", [128, 128], BF, isOutput=False)
    ident_in = nc.declare_dram_parameter("ident", [128, 128], BF, isOutput=False)
    idxw = nc.declare_dram_parameter("idxw", [128, NG * 8 * CQ], mybir.dt.int16, isOutput=False)
    rrval = nc.declare_dram_parameter("rrval", [128, NG * 2 * CQ], BF, isOutput=False)
    gcnt = nc.declare_dram_parameter("gcnt", [128, NG], mybir.dt.int32, isOutput=False)
    outT = nc.declare_dram_parameter("outT", [2, 128, c.RPC], F32, isOutput=True)

    htabs = [nc.dram_tensor(f"htab{i}", [SEG, HID], BF) for i in range(4)]
    cc1i = nc.dram_tensor("cc1i", [128, 4], F32)
    cc1o = nc.dram_tensor("cc1o", [128, 4], F32, addr_space="Shared")
    cc2i = nc.dram_tensor("cc2i", [128, 4], F32)
    cc2o = nc.dram_tensor("cc2o", [128, 4], F32, addr_space="Shared")

    from contextlib import ExitStack
    st_ctx = ExitStack()
    T = lambda name, shape, dt: st_ctx.enter_context(nc.sbuf_tensor(name, shape, dt))
    P = lambda name, shape, dt=F32: st_ctx.enter_context(nc.psum_tensor(name, shape, dt))
    S = lambda name: st_ctx.enter_context(nc.semaphore(name))

    with st_ctx:
        xts = T("xts", [128, c.R_XT, 4, 512], BF)
        wg = T("wg", [128, 4, HID], BF)
        w1s = T("w1s", [128, 2, 2, 128], BF)
        w2s = T("w2s", [128, 2, 2, 128], BF)
        ht = T("ht", [128, 2, 4, HID], BF)
        ev1 = T("ev1", [128, 2, HID], BF)
        gr = T("gr", [128, 8, CQ, HID], BF)
        ss = T("ss", [128, 6, CQ, 128], BF)
        ixs = T("ixs", [128, c.R_IXB, c.IB, 8 * CQ], mybir.dt.int16)
        rvs = T("rvs", [128, c.R_RVB, c.IB, 2 * CQ], BF)
        cnt = T("cnt", [128, NG], mybir.dt.int32)
        io_sb = T("io_sb", [128, 128], BF)
        id_sb = T("id_sb", [128, 128], BF)
        hat = T("hat", [128, 2, c.RPC], BF)
        h1 = T("h1", [128, 2, c.RPC], BF)
        bg = T("bg", [128, 2], F32)
        g1s = T("g1s", [128, 2], F32)
        be1s = T("be1s", [128, 2], F32)
        g2s = T("g2s", [128, 2], F32)
        be2s = T("be2s", [128, 2], F32)
        stt = T("stt", [128, 2, 2, c.NCH], F32)
        ccp = T("ccp", [128, 4], F32)
        gst = T("gst", [128, 8], F32)
        kdt = T("kdt", [128, 16], F32)
        kd1 = T("kd1", [128, 4], F32)
        kd2 = T("kd2", [128, 4], F32)
        ot = T("ot", [128, 2, 512], F32)

        pa = [P("pa0", [128, HID]), P("pa1", [128, HID])]
        pb = [P("pb0", [128, HID]), P("pb1", [128, HID])]
        pt = [P("pt0", [128, 2, 128], BF), P("pt1", [128, 2, 128], BF)]
        p3 = [P(f"p3{i}", [128, 512]) for i in range(2)]

        s_pre = S("s_pre"); s_ms = S("s_ms")
        s_slab = [S(f"s_slab{i}") for i in range(c.R_XT)]
        s_p1ps = S("s_p1ps"); s_p1ev = S("s_p1ev")
        s_p1st = [S(f"s_p1st{i}") for i in range(2)]
        s_idx = [S(f"s_idx{i}") for i in range(c.R_IXB)]
        s_rv = [S(f"s_rv{i}") for i in range(c.R_RVB)]
        s_g = [S(f"s_g{i}") for i in range(8)]
        s_s = S("s_s")
        s_pg = S("s_pg"); s_e1 = S("s_e1"); s_pt = S("s_pt"); s_hacc = S("s_hacc")
        s_3ps = S("s_3ps"); s_3c = S("s_3c"); s_sq = S("s_sq"); s_h1 = S("s_h1"); s_oev = S("s_oev")
        s_ost = [S(f"s_ost{i}") for i in range(2)]
        s_stf = S("s_stf"); s_cio = S("s_cio"); s_cc = S("s_cc")
        s_kd = S("s_kd")

        N_PRE = 11 * 16
        batch_rgs = [min(4, c.RG - 4 * b) for b in range(c.HB)]
        # phase-3 chunk rows
        rows_t = [min(512, c.RPC - 512 * t) for t in range(c.NCH)]
        rreal_t = [max(0, min(rows_t[t], c.ROWS_REAL - 512 * t)) for t in range(c.NCH)]
        assert all(r > 0 for r in rreal_t)
        NT = 2 * c.NCH          # tiles per phase-3 pass

        def gq(gi):
            return divmod(gi, WPC)  # (quarter, window)

        with nc.Block() as block:

            @block.gpsimd
            def _(g: bass.BassGpSimd):
                g.load_library(mlp)
                # ---- preloads (11 DMAs; casts ride SWDGE) ----
                g.dma_start(wg[:, :, :], _ap(wgcn, 0, [[HID, 128], [128 * HID, 4], [1, HID]])).then_inc(s_pre, 16)
                g.dma_start(w1s[:, :, :, :], _ap(w1, 0, [[HID, 128], [128 * HID, 2], [128, 2], [1, 128]])).then_inc(s_pre, 16)
                g.dma_start(w2s[:, :, :, :], _ap(w2, 0, [[HID, 128], [128 * HID, 2], [128, 2], [1, 128]])).then_inc(s_pre, 16)
                g.dma_start(io_sb[:, :], iota_in[:, :]).then_inc(s_pre, 16)
                g.dma_start(id_sb[:, :], ident_in[:, :]).then_inc(s_pre, 16)
                g.dma_start(bg[:, :], bgcn2[:, :]).then_inc(s_pre, 16)
                g.dma_start(g1s[:, :], g1v[:, :]).then_inc(s_pre, 16)
                g.dma_start(be1s[:, :], be1v[:, :]).then_inc(s_pre, 16)
                g.dma_start(g2s[:, :], g2v[:, :]).then_inc(s_pre, 16)
                g.dma_start(be2s[:, :], be2v[:, :]).then_inc(s_pre, 16)
                g.dma_start(cnt[:, :], gcnt[:, :]).then_inc(s_pre, 16)
                # ---- phase 2: quarter-gated gathers (4 SWDGE queues) ----
                g.wait_ge(s_ms, 8)
                with g.register("cntreg") as creg:
                    for q in range(Q):
                        nb = c.QSL * (q + 1)  # h-store batches needed
                        g.wait_ge(s_p1st[0], 16 * ((nb + 1) // 2))
                        g.wait_ge(s_p1st[1], 16 * (nb // 2))
                        for w in range(WPC):
                            gi = q * WPC + w
                            ib = gi // c.IB
                            g.wait_ge(s_idx[ib % c.R_IXB], 16 * (ib // c.R_IXB + 1))
                            if gi >= 8:
                                g.wait_ge(s_pg, gi - 7)
                            g.reg_load(creg, _ap(cnt, gi, [[NG, 1], [1, 1]]))
                            g.dma_gather(
                                gr[:, gi % 8, :, :],
                                htabs[q][:, :],
                                ixs[:, ib % c.R_IXB, gi % c.IB, :],
                                c.GSLOT, creg, HID, single_packet=False,
                                queue_num=w % 4,
                            ).then_inc(s_g[gi % 8], 16)
                # ---- phase 3: stats AllReduce x2 ----
                g.wait_ge(s_stf, 1)
                g.dma_start(cc1i[:, :], ccp[:, :]).then_inc(s_cio, 16)
                g.wait_ge(s_cio, 16)
                g.collective_compute("AllReduce", OP.add, replica_groups=[list(range(c.CORES))],
                                     ins=[cc1i.ap().opt()], outs=[cc1o.ap().opt()]).then_inc(s_cc, 1)
                g.wait_ge(s_cc, 1)
                g.dma_start(gst[:, 0:4], cc1o[:, :]).then_inc(s_cio, 16)
                g.wait_ge(s_stf, 2)
                g.dma_start(cc2i[:, :], ccp[:, :]).then_inc(s_cio, 16)
                g.wait_ge(s_cio, 48)
                g.collective_compute("AllReduce", OP.add, replica_groups=[list(range(c.CORES))],
                                     ins=[cc2i.ap().opt()], outs=[cc2o.ap().opt()]).then_inc(s_cc, 1)
                g.wait_ge(s_cc, 2)
                g.dma_start(gst[:, 4:8], cc2o[:, :]).then_inc(s_cio, 16)
                g.wait_ge(s_cio, 64)

            @block.sync
            def _(sp):
                # phase-1 slab loads + h-table stores (pure bf16 copies) and
                # idx/rrval streams, statically interleaved.
                def slab(sl):
                    if sl >= c.R_XT:
                        sp.wait_ge(s_p1ps, 4 * (sl - c.R_XT) + 4)
                    sp.dma_start(
                        xts[:, sl % c.R_XT, :, :],
                        _ap(xT, 512 * sl, [[c.TABROWS, 128], [128 * c.TABROWS, 4], [1, 512]]),
                    ).then_inc(s_slab[sl % c.R_XT], 16)

                def store(b):
                    nt = batch_rgs[b]
                    sp.wait_ge(s_p1ev, min(4 * b + nt, c.RG))
                    qb, lb = divmod(b, c.QSL)
                    sp.dma_start(
                        _ap(htabs[qb], lb * 512 * HID, [[HID, 128], [128 * HID, nt], [1, HID]]),
                        ht[:, b % 2, 0:nt, :],
                    ).then_inc(s_p1st[b % 2], 16)

                def idx_stream(ib):
                    if ib >= c.R_IXB:
                        sp.wait_ge(s_pg, c.IB * (ib - c.R_IXB) + c.IB)
                    w8 = c.IB * 8 * CQ
                    sp.dma_start(ixs[:, ib % c.R_IXB, :, :],
                                 idxw[:, ib * w8:(ib + 1) * w8]).then_inc(s_idx[ib % c.R_IXB], 16)

                def rv_stream(rb):
                    if rb >= c.R_RVB:
                        sp.wait_ge(s_s, CQ * c.IB * (rb - c.R_RVB + 1))
                    w2 = c.IB * 2 * CQ
                    sp.dma_start(rvs[:, rb % c.R_RVB, :, :],
                                 rrval[:, rb * w2:(rb + 1) * w2]).then_inc(s_rv[rb % c.R_RVB], 16)

                # section A: quarter-0 slabs/stores + pre-streams
                # interleave: idx and rv alternating so both rings fill early
                pre = [x for pair in zip(
                    [("i", b) for b in range(c.R_IXB)],
                    [("r", b) for b in range(c.R_IXB)]) for x in pair] + \
                    [("r", b) for b in range(c.R_IXB, c.R_RVB)]
                pi = 0
                for sl in range(c.QSL):
                    slab(sl)
                    for _k in range(3):
                        if pi < len(pre):
                            kind, gg = pre[pi]; pi += 1
                            (idx_stream if kind == "i" else rv_stream)(gg)
                    if sl >= 2:
                        store(sl - 2)
                while pi < len(pre):
                    kind, gg = pre[pi]; pi += 1
                    (idx_stream if kind == "i" else rv_stream)(gg)
                for b in range(max(0, c.QSL - 2), c.QSL):
                    store(b)
                # sections q=0..3: streams ahead + next quarter's slabs/stores
                for q in range(Q):
                    base_sl = c.QSL * (q + 1)
                    if q < 3:
                        for s0 in range(min(2, c.QSL)):
                            slab(base_sl + s0)
                    for w in range(WPC):
                        gsl = q * WPC + w
                        if q < 3:
                            if w % 2 == 0 and w // 2 + 2 < c.QSL:
                                slab(base_sl + 2 + w // 2)
                            elif w % 2 == 1 and (w - 1) // 2 < c.QSL:
                                store(base_sl + (w - 1) // 2)
                        # streams emitted 2-3 batches late so their ring-
                        # protect waits trail PE progress with slack
                        if gsl % c.IB == 0 and gsl >= 16:
                            ib = c.R_IXB + (gsl - 16) // c.IB
                            if ib < c.NB:
                                idx_stream(ib)
                        if gsl % c.IB == 4 and gsl >= 28:
                            rb = c.R_RVB + (gsl - 28) // c.IB
                            if rb < c.NB:
                                rv_stream(rb)
                # phase 3 out stores
                for i in range(NT):
                    hf, t = divmod(i, c.NCH)
                    sp.wait_ge(s_oev, i + 1)
                    sp.dma_start(
                        _ap(outT, hf * 128 * c.RPC + t * 512, [[c.RPC, 128], [1, rows_t[t]]]),
                        ot[:, i % 2, 0:rows_t[t]],
                    ).then_inc(s_ost[i % 2], 16)
                sp.wait_ge(s_ost[0], 16 * ((NT + 1) // 2))
                sp.wait_ge(s_ost[1], 16 * (NT // 2))

            @block.tensor
            def _(pe: bass.BassTensorEngine):
                pe.wait_ge(s_pre, N_PRE)

                def rg_mm(rg):
                    sl = rg // 4
                    pe.wait_ge(s_slab[sl % c.R_XT], 16 * (sl // c.R_XT + 1))
                    if rg >= 2:
                        pe.wait_ge(s_p1ev, rg - 1)
                    j = rg % 4
                    for kc in range(4):
                        mm = pe.matmul(
                            pa[rg % 2][:, :],
                            xts[:, (rg // 4) % c.R_XT, kc, 128 * j:128 * (j + 1)],
                            wg[:, kc, :],
                            start=(kc == 0), stop=(kc == 3),
                        )
                        if kc == 3:
                            mm.then_inc(s_p1ps, 1)

                def transpose(v):
                    pe.wait_ge(s_e1, v + 1)
                    if v >= 2:
                        pe.wait_ge(s_hacc, v - 1)
                    for i in range(2):
                        pe.matmul(pt[v % 2][:, i, :], ev1[:, v % 2, 128 * i:128 * (i + 1)],
                                  id_sb[:, :], is_transpose=True, start=True, stop=True).then_inc(s_pt, 1)

                def group_mm(gi):
                    pe.wait_ge(s_g[gi % 8], 16 * (gi // 8 + 1))
                    pe.wait_ge(s_s, CQ * (gi + 1))
                    if gi >= 2:
                        pe.wait_ge(s_e1, gi - 1)
                    for ch in range(CQ):
                        mm = pe.matmul(
                            pb[gi % 2][:, :],
                            ss[:, gi % 6, ch, :],
                            gr[:, gi % 8, ch, :],
                            start=(ch == 0), stop=(ch == CQ - 1),
                        )
                        if ch == CQ - 1:
                            mm.then_inc(s_pg, 1)
                    if gi >= 1:
                        transpose(gi - 1)

                for rg in range(c.QRG):
                    rg_mm(rg)
                for q in range(Q):
                    for w in range(WPC):
                        gi = q * WPC + w
                        group_mm(gi)
                        if q < 3:
                            rg_mm(c.QRG * (q + 1) + 2 * w)
                            rg_mm(c.QRG * (q + 1) + 2 * w + 1)
                transpose(NG - 1)
                # ---- phase 3: 4 passes x (2 halves x NCH chunks) ----
                pe.wait_ge(s_hacc, NG)
                for i in range(4 * NT):
                    p, j = divmod(i, NT)
                    hf, t = divmod(j, c.NCH)
                    if i >= 2:
                        pp, jj = divmod(i - 2, NT)
                        if pp == 0:
                            pe.wait_ge(s_sq, jj + 1)
                        elif pp == 1:
                            pe.wait_ge(s_h1, jj + 1)
                        elif pp == 2:
                            pe.wait_ge(s_sq, NT + jj + 1)
                        else:
                            pe.wait_ge(s_oev, jj + 1)
                    if p == 2:
                        pe.wait_ge(s_h1, c.NCH + t + 1)
                    ws = w1s if p < 2 else w2s
                    src = hat if p < 2 else h1
                    rt = rows_t[t]
                    for ci in range(2):
                        mm = pe.matmul(
                            p3[i % 2][:, 0:rt],
                            ws[:, ci, hf, :],
                            src[:, ci, 512 * t:512 * t + rt],
                            start=(ci == 0), stop=(ci == 1),
                        )
                        if ci == 1:
                            mm.then_inc(s_3ps, 1)

            @block.vector
            def _(v: bass.BassVectorEngine):
                for sl8 in range(8):
                    v.memset(gr[:, sl8, :, :], 0).then_inc(s_ms, 1)
                v.wait_ge(s_pre, N_PRE)
                # ---- phase 2: S-builds + hat accumulation ----
                RVSZ = c.R_RVB * c.IB * 2 * CQ
                io_b = _ap(io_sb, 0, [[128, 128], [0, CQ], [1, 128]])

                def build(gi):
                    rb = gi // c.IB
                    v.wait_ge(s_rv[rb % c.R_RVB], 16 * (rb // c.R_RVB + 1))
                    if gi >= 6:
                        v.wait_ge(s_pg, gi - 5)
                    rbase = ((rb % c.R_RVB) * c.IB + gi % c.IB) * 2 * CQ
                    rr_b = _ap(rvs, rbase, [[RVSZ, 128], [2, CQ], [0, 128]])
                    val_b = _ap(rvs, rbase + 1, [[RVSZ, 128], [2, CQ], [0, 128]])
                    v.scalar_tensor_tensor(
                        ss[:, gi % 6, :, :], io_b, 0.0, rr_b,
                        OP.bypass, OP.is_equal,
                    )
                    v.tensor_mul(ss[:, gi % 6, :, :], ss[:, gi % 6, :, :], val_b).then_inc(s_s, CQ)

                HATSZ = 2 * c.RPC

                def acc(vv):
                    qv, wv = gq(vv)
                    v.wait_ge(s_pt, 2 * (vv + 1))
                    hat_full = _ap(hat, 128 * wv, [[HATSZ, 128], [c.RPC, 2], [1, 128]])
                    pt_full = pt[vv % 2][:, :, :]
                    if qv == 0:
                        v.tensor_scalar(hat_full, pt_full, 0.0, None, OP.add).then_inc(s_hacc, 1)
                    elif qv < 3:
                        v.tensor_add(hat_full, pt_full, hat_full).then_inc(s_hacc, 1)
                    else:
                        for i in range(2):
                            op = v.scalar_tensor_tensor(
                                hat[:, i, 128 * wv:128 * (wv + 1)],
                                pt[vv % 2][:, i, :], bg[:, i:i + 1],
                                hat[:, i, 128 * wv:128 * (wv + 1)],
                                OP.add, OP.add,
                            )
                            if i == 1:
                                op.then_inc(s_hacc, 1)

                for gi in range(NG):
                    build(gi)
                    if gi >= 1:
                        acc(gi - 1)
                acc(NG - 1)
                # ---- phase 3 ----
                for layer in range(2):
                    base = 0 if layer == 0 else 2 * NT
                    for j in range(NT):
                        hf, t = divmod(j, c.NCH)
                        v.wait_ge(s_3ps, base + j + 1)
                        rr = rreal_t[t]
                        psl = p3[(base + j) % 2]
                        v.tensor_reduce(stt[:, hf, 0, t:t + 1], psl[:, 0:rr],
                                        mybir.AxisListType.X, OP.add).then_inc(s_3c, 1)
                    v.wait_ge(s_sq, NT * (layer + 1))
                    v.drain()
                    v.tensor_reduce(ccp[:, 0:1], stt[:, 0, 0, :], mybir.AxisListType.X, OP.add)
                    v.tensor_reduce(ccp[:, 1:2], stt[:, 0, 1, :], mybir.AxisListType.X, OP.add)
                    v.tensor_reduce(ccp[:, 2:3], stt[:, 1, 0, :], mybir.AxisListType.X, OP.add)
                    v.tensor_reduce(ccp[:, 3:4], stt[:, 1, 1, :], mybir.AxisListType.X, OP.add)
                    v.drain().then_inc(s_stf, 1)
                    # finalize after AllReduce
                    v.wait_ge(s_cio, 32 + 32 * layer)
                    gof = 4 * layer
                    sums = _ap(gst, gof, [[8, 128], [2, 2]])
                    sqs = _ap(gst, gof + 1, [[8, 128], [2, 2]])
                    inv_n = 1.0 / c.N
                    v.tensor_scalar(kdt[:, 0:2], sums, inv_n, None, OP.mult)
                    v.tensor_scalar(kdt[:, 2:4], sqs, inv_n, None, OP.mult)
                    v.drain()
                    v.tensor_mul(kdt[:, 4:6], kdt[:, 0:2], kdt[:, 0:2])
                    v.drain()
                    v.tensor_sub(kdt[:, 6:8], kdt[:, 2:4], kdt[:, 4:6])
                    v.drain()
                    v.tensor_scalar(kdt[:, 6:8], kdt[:, 6:8], c.EPS, None, OP.add)
                    v.drain().then_inc(s_kd, 1)
                    v.wait_ge(s_kd, 2 + 3 * layer)
                    v.reciprocal(kdt[:, 10:12], kdt[:, 8:10])
                    v.drain()
                    kd = kd1 if layer == 0 else kd2
                    gv = g1s if layer == 0 else g2s
                    bev = be1s if layer == 0 else be2s
                    v.tensor_mul(kd[:, 0:2], gv[:, :], kdt[:, 10:12])
                    v.drain()
                    v.tensor_mul(kdt[:, 12:14], kdt[:, 0:2], kd[:, 0:2])
                    v.drain()
                    v.tensor_sub(kd[:, 2:4], bev[:, :], kdt[:, 12:14])
                    v.drain().then_inc(s_kd, 1)

            @block.scalar
            def _(a: bass.BassScalarEngine):
                a.wait_ge(s_pre, N_PRE)

                def rg_ev(rg):
                    a.wait_ge(s_p1ps, rg + 1)
                    b = rg // 4
                    if b >= 2 and rg % 4 == 0:
                        a.wait_ge(s_p1st[b % 2], 16 * ((b - 2) // 2 + 1))
                    a.activation(ht[:, b % 2, rg % 4, :], pa[rg % 2][:, :], AF.Identity).then_inc(s_p1ev, 1)

                def group_ev(gi):
                    a.wait_ge(s_pg, gi + 1)
                    if gi >= 2:
                        a.wait_ge(s_pt, 2 * (gi - 1))
                    a.activation(ev1[:, gi % 2, :], pb[gi % 2][:, :], AF.Identity).then_inc(s_e1, 1)

                for rg in range(c.QRG):
                    rg_ev(rg)
                for q in range(Q):
                    for w in range(WPC):
                        gi = q * WPC + w
                        group_ev(gi)
                        if q < 3:
                            rg_ev(c.QRG * (q + 1) + 2 * w)
                            rg_ev(c.QRG * (q + 1) + 2 * w + 1)
                # ---- phase 3 ----
                for layer in range(2):
                    sbase = 0 if layer == 0 else 2 * NT
                    for j in range(NT):
                        hf, t = divmod(j, c.NCH)
                        a.wait_ge(s_3ps, sbase + j + 1)
                        a.wait_ge(s_3c, NT * layer + j + 1)
                        rr = rreal_t[t]
                        psl = p3[(sbase + j) % 2]
                        a.activation(psl[:, 0:rr], psl[:, 0:rr], AF.Square,
                                     accum_out=stt[:, hf, 1, t:t + 1]).then_inc(s_sq, 1)
                    # sqrt step for k/d
                    a.wait_ge(s_kd, 1 + 3 * layer)
                    a.sqrt(kdt[:, 8:10], kdt[:, 6:8]).then_inc(s_kd, 1)
                    a.wait_ge(s_kd, 3 + 3 * layer)
                    kd = kd1 if layer == 0 else kd2
                    pbase = NT if layer == 0 else 3 * NT
                    for j in range(NT):
                        hf, t = divmod(j, c.NCH)
                        a.wait_ge(s_3ps, pbase + j + 1)
                        rt = rows_t[t]
                        psl = p3[(pbase + j) % 2]
                        if layer == 0:
                            a.activation(h1[:, hf, 512 * t:512 * t + rt], psl[:, 0:rt], AF.Relu,
                                         bias=kd[:, 2 + hf:3 + hf], scale=kd[:, hf:hf + 1]).then_inc(s_h1, 1)
                        else:
                            if j >= 2:
                                a.wait_ge(s_ost[j % 2], 16 * ((j - 2) // 2 + 1))
                            a.activation(ot[:, j % 2, 0:rt], psl[:, 0:rt], AF.Identity,
                                         bias=kd[:, 2 + hf:3 + hf], scale=kd[:, hf:hf + 1]).then_inc(s_oev, 1)

        nc.compile()
    return nc


# ---------------------------------------------------------------------------
# host-side preprocessing
# ---------------------------------------------------------------------------

def preprocess(x, edge_row, edge_col, edge_val, c: Cfg):
    N, E, WPC, Q, SEG = c.N, len(edge_row), c.WPC, c.Q, c.SEG
    deg = np.bincount(edge_row, minlength=N)
    order = np.argsort(-deg, kind="stable")
    rank = np.empty(N, np.int64)
    rank[order] = np.arange(N)
    core = (rank % c.CORES).astype(np.int32)

    win_of = np.empty(N, np.int32)
    slot_of = np.empty(N, np.int32)
    caps = np.full(WPC, 128, np.int32)
    tail = c.ROWS_REAL - 128 * (WPC - 1)
    caps[WPC - 1] = tail if tail > 0 else 128
    assert caps.sum() >= c.ROWS_REAL
    for k in range(c.CORES):
        nodes = order[k::c.CORES]
        heap = [(0, w) for w in range(WPC)]
        heapq.heapify(heap)
        fill = np.zeros(WPC, np.int32)
        for n in nodes:
            s, w = heapq.heappop(heap)
            win_of[n] = w
            slot_of[n] = fill[w]
            fill[w] += 1
            if fill[w] < caps[w]:
                heapq.heappush(heap, (s + int(deg[n]), w))
    ek = core[edge_row]
    ew = win_of[edge_row]
    er = slot_of[edge_row]
    eq = (edge_col // SEG).astype(np.int64)
    erel = (edge_col - eq * SEG).astype(np.int16)
    # q-major bucket key: (core, quarter, window)
    key = ((ek.astype(np.int64) * Q + eq) * WPC + ew)
    sidx = np.argsort(key, kind="stable")
    key_s = key[sidx]
    ngroups = c.CORES * c.NG
    counts = np.bincount(key_s, minlength=ngroups)
    cnt128 = np.maximum((counts + 127) // 128, 1)
    cq_needed = int(cnt128.max())
    if cq_needed > c.CQ:
        return None, cq_needed  # caller rebuilds with larger CQ
    GSLOT = c.GSLOT
    starts = np.zeros(ngroups, np.int64)
    starts[1:] = np.cumsum(counts)[:-1]
    pos = np.arange(E) - starts[key_s]
    gslot = key_s * GSLOT + pos
    TOT = ngroups * GSLOT
    idx_sl = np.full(TOT, -1, np.int16)
    rr_sl = np.zeros(TOT, np.float32)
    val_sl = np.zeros(TOT, np.float32)
    idx_sl[gslot] = erel[sidx]
    rr_sl[gslot] = er[sidx].astype(np.float32)
    val_sl[gslot] = np.asarray(edge_val, np.float32)[sidx]
    sig = np.arange(TOT, dtype=np.int64) % GSLOT
    gof = np.arange(TOT, dtype=np.int64) // GSLOT
    padmask = (sig >= counts[gof]) & (sig < cnt128[gof] * 128)
    idx_sl[padmask] = 0
    gcnt_all = (cnt128 * 128).astype(np.int32)

    xTp = np.zeros((c.IN_C, c.TABROWS), dtype=BF16)
    xTp[:, :N] = np.asarray(x, np.float32).T.astype(BF16)

    per_core = []
    idx_c = idx_sl.reshape(c.CORES, c.NG, GSLOT)
    rr_c = rr_sl.reshape(c.CORES, c.NG, c.CQ, 128)
    val_c = val_sl.reshape(c.CORES, c.NG, c.CQ, 128)
    for k in range(c.CORES):
        # per-group 16-wrap: idx j -> (partition j%16, col g*8CQ + j//16)
        a = idx_c[k].reshape(c.NG, 8 * c.CQ, 16)
        w16 = np.transpose(a, (2, 0, 1)).reshape(16, c.NG * 8 * c.CQ)
        idxw_k = np.ascontiguousarray(np.tile(w16, (8, 1)))
        # per-group rr/val: [128, NG, CQ, 2] -> [128, NG*2CQ]
        rrT = np.transpose(rr_c[k], (2, 0, 1))   # [128, NG, CQ]
        valT = np.transpose(val_c[k], (2, 0, 1))
        rrval_k = np.ascontiguousarray(
            np.stack([rrT, valT], axis=-1).reshape(128, -1)).astype(BF16)
        per_core.append(dict(idxw=idxw_k, rrval=rrval_k,
                             gcnt=np.ascontiguousarray(
                                 np.tile(gcnt_all.reshape(c.CORES, -1)[k:k + 1], (128, 1)))))
    meta = dict(core=core, win_of=win_of, slot_of=slot_of, xTp=xTp)
    return (per_core, meta), None


def make_in_maps(inputs, c: Cfg):
    res, cq_needed = preprocess(inputs["x"], np.asarray(inputs["edge_row"]),
                                np.asarray(inputs["edge_col"]), np.asarray(inputs["edge_val"]), c)
    if res is None:
        return None, cq_needed
    per_core, meta = res
    iota = np.broadcast_to(np.arange(128, dtype=np.float32), (128, 128)).astype(BF16)
    ident = np.eye(128, dtype=np.float32).astype(BF16)

    def v2(b):
        return np.ascontiguousarray(np.asarray(b, np.float32).reshape(2, 128).T)

    shared = dict(
        xT=meta["xTp"],
        wgcn=np.asarray(inputs["W_gcn"], np.float32),
        w1=np.asarray(inputs["W1"], np.float32),
        w2=np.asarray(inputs["W2"], np.float32),
        bgcn2=v2(inputs["b_gcn"]), g1v=v2(inputs["g1"]), be1v=v2(inputs["be1"]),
        g2v=v2(inputs["g2"]), be2v=v2(inputs["be2"]),
        iota=np.ascontiguousarray(iota), ident=np.ascontiguousarray(ident),
    )
    in_maps = [dict(shared, **pc) for pc in per_core]
    return (in_maps, meta), None


def unshard(results, meta, c: Cfg):
    core, win_of, slot_of = meta["core"], meta["win_of"], meta["slot_of"]
    out = np.empty((c.N, c.OUT_C), np.float32)
    rowpos = win_of.astype(np.int64) * 128 + slot_of
    for k in range(c.CORES):
        o = np.asarray(results[k]["outT"]).reshape(2, 128, c.RPC)
        o = np.transpose(o, (2, 0, 1)).reshape(c.RPC, c.OUT_C)
        nodes_k = np.flatnonzero(core == k)
        out[nodes_k] = o[rowpos[nodes_k]]
    return out


_NC_CACHE = {}


def get_nc(c: Cfg):
    key = (c.N, c.SEG, c.WPC, c.CQ, c.PH)
    if key not in _NC_CACHE:
        _NC_CACHE[key] = build(c)
    return _NC_CACHE[key]


def kernel(**inputs):
    c = Cfg()
    while True:
        res, cq_needed = make_in_maps(inputs, c)
        if res is not None:
            break
        c = Cfg(CQ=cq_needed)
    in_maps, meta = res
    nc = get_nc(c)
    r = run_bass_kernel_spmd(nc, in_maps, list(range(c.CORES)))
    return unshard(r.results, meta, c)


# revision 27
# speedup vs baseline: 1.1944x; 1.1944x over previous
"""Trainium2 Bass kernel for nn_Encoder (GCN layer + MLP/BatchNorm), 8 NeuronCores.

Strategy (per core, SPMD over 8 cores):
  Phase 1 (replicated): h = x @ W_gcn written as a row-major DRAM table
    [Q*SEG, 256] bf16.  x arrives host-transposed AND pre-cast to bf16
    (xT [512, Q*SEG]), so slab loads / h-table stores are pure copies that
    ride the sync-queue hwdge, leaving the gpsimd queue free for gathers.
  Phase 2 (sharded by destination row): edges are host-bucketed by
    (source quarter, dest window of 128 rows) in QUARTER-MAJOR order.  As
    soon as quarter q's table stores land, its 98 window-gathers start
    (gpsimd dma_gather, 4 SWDGE queues round-robin, 8-deep gr ring).  A
    batched DVE op builds S^T[j,r] = val_j * (rowrel_j == r) per group; PE
    matmuls accumulate the quarter-partial into PSUM, which is transposed
    (PE) and DVE-accumulated into the column-major hat buffer.  Remaining
    phase-1 row-groups are statically interleaved into the PE/scalar
    streams during sweeps 0-2, overlapping phase 1 under the gathers.
  Phase 3: z1 = W1-matmuls (W1 stationary), BatchNorm stats reduced
    locally and AllReduce'd across the 8 cores, window recomputed and
    Relu+affine applied in one ACT op; same for W2 / BN2, final affine
    written f32 to outT [2,128,RPC].

Host side does only index/layout work: degree-balanced node->window
assignment (LPT), edge bucketing/sorting, padding, and the output unpermute.
"""

import heapq
import numpy as np
import ml_dtypes

import concourse.bacc as bacc
from concourse import bass, mybir
from concourse.bass_utils import run_bass_kernel_spmd
from concourse.library_config import mlp

BF16 = ml_dtypes.bfloat16
F32 = mybir.dt.float32
BF = mybir.dt.bfloat16
AF = mybir.ActivationFunctionType
OP = mybir.AluOpType
RACE_DETECT = True


class Cfg:
    def __init__(self, N=100000, E=3200000, SEG=25088, WPC=98, CQ=10, PH=3):
        self.N, self.E, self.SEG, self.WPC, self.CQ = N, E, SEG, WPC, CQ
        self.PH = PH
        self.CORES = 8
        self.Q = 4
        self.IN_C, self.HID, self.OUT_C = 512, 256, 256
        self.EPS = 1e-5
        self.TABROWS = self.Q * SEG          # h-table rows (>= N, %128 == 0)
        assert self.TABROWS >= N and self.TABROWS % 128 == 0
        assert SEG <= 32767 and SEG % 512 == 0  # quarter tensors batch-aligned
        self.RG = self.TABROWS // 128        # phase-1 row groups
        self.SL = (self.TABROWS + 511) // 512  # phase-1 slabs
        self.HB = (self.RG + 3) // 4         # phase-1 h-store batches of 4 rgs
        self.QSL = SEG // 512                # slabs per quarter
        self.QRG = self.RG // 4              # row groups per quarter
        assert self.SL == self.HB == 4 * self.QSL and self.RG == 4 * self.QRG
        assert self.QRG == 2 * WPC           # PE interleave covers exactly
        self.RPC = WPC * 128                 # rows per core (padded)
        self.ROWS_REAL = N // self.CORES     # real rows per core
        assert self.ROWS_REAL <= self.RPC
        self.NCH = (self.RPC + 511) // 512   # phase-3 row chunks
        self.GSLOT = 128 * CQ                # slots per (quarter, window) group
        self.NG = WPC * self.Q               # gather groups per core (q-major)
        # rings
        self.R_XT = 3                        # xts slab ring
        self.IB = 8                          # groups per idx/rv stream batch
        self.NB = (self.NG + self.IB - 1) // self.IB
        assert self.NG % self.IB == 0
        self.R_IXB = min(6, self.NB)         # idx stream batch ring
        self.R_RVB = min(8, self.NB)         # rrval stream batch ring


def _ap(t, off, pattern):
    return bass.AP(t, off, pattern)


def build(c: Cfg):
    nc = bacc.Bacc("TRN2", debug=False, num_swdge_queues=4, detect_race_conditions=RACE_DETECT)
    CQ, Q, WPC, SEG, HID, NG = c.CQ, c.Q, c.WPC, c.SEG, c.HID, c.NG

    xT = nc.declare_dram_parameter("xT", [c.IN_C, c.TABROWS], BF, isOutput=False)
    wgcn = nc.declare_dram_parameter("wgcn", [c.IN_C, HID], F32, isOutput=False)
    w1 = nc.declare_dram_parameter("w1", [HID, HID], F32, isOutput=False)
    w2 = nc.declare_dram_parameter("w2", [HID, c.OUT_C], F32, isOutput=False)
    bgcn2 = nc.declare_dram_parameter("bgcn2", [128, 2], F32, isOutput=False)
    g1v = nc.declare_dram_parameter("g1v", [128, 2], F32, isOutput=False)
    be1v = nc.declare_dram_parameter("be1v", [128, 2], F32, isOutput=False)
    g2v = nc.declare_dram_parameter("g2v", [128, 2], F32, isOutput=False)
    be2v = nc.declare_dram_parameter("be2v", [128, 2], F32, isOutput=False)
    iota_in = nc.declare_dram_parameter("iota", [128, 128], BF, isOutput=False)
    ident_in = nc.declare_dram_parameter("ident", [128, 128], BF, isOutput=False)
    idxw = nc.declare_dram_parameter("idxw", [128, NG * 8 * CQ], mybir.dt.int16, isOutput=False)
    rrval = nc.declare_dram_parameter("rrval", [128, NG * 2 * CQ], BF, isOutput=False)
    gcnt = nc.declare_dram_parameter("gcnt", [128, NG], mybir.dt.int32, isOutput=False)
    outT = nc.declare_dram_parameter("outT", [2, 128, c.RPC], F32, isOutput=True)

    htabs = [nc.dram_tensor(f"htab{i}", [SEG, HID], BF) for i in range(4)]
    cc1i = nc.dram_tensor("cc1i", [128, 4], F32)
    cc1o = nc.dram_tensor("cc1o", [128, 4], F32, addr_space="Shared")
    cc2i = nc.dram_tensor("cc2i", [128, 4], F32)
    cc2o = nc.dram_tensor("cc2o", [128, 4], F32, addr_space="Shared")

    from contextlib import ExitStack
    st_ctx = ExitStack()
    T = lambda name, shape, dt: st_ctx.enter_context(nc.sbuf_tensor(name, shape, dt))
    P = lambda name, shape, dt=F32: st_ctx.enter_context(nc.psum_tensor(name, shape, dt))
    S = lambda name: st_ctx.enter_context(nc.semaphore(name))

    with st_ctx:
        xts = T("xts", [128, c.R_XT, 4, 512], BF)
        wg = T("wg", [128, 4, HID], BF)
        w1s = T("w1s", [128, 2, 2, 128], BF)
        w2s = T("w2s", [128, 2, 2, 128], BF)
        ht = T("ht", [128, 2, 4, HID], BF)
        ev1 = T("ev1", [128, 2, HID], BF)
        gr = T("gr", [128, 8, CQ, HID], BF)
        ss = T("ss", [128, 6, CQ, 128], BF)
        ixs = T("ixs", [128, c.R_IXB, c.IB, 8 * CQ], mybir.dt.int16)
        rvs = T("rvs", [128, c.R_RVB, c.IB, 2 * CQ], BF)
        cnt = T("cnt", [128, NG], mybir.dt.int32)
        io_sb = T("io_sb", [128, 128], BF)
        id_sb = T("id_sb", [128, 128], BF)
        hat = T("hat", [128, 2, c.RPC], BF)
        h1 = T("h1", [128, 2, c.RPC], BF)
        bg = T("bg", [128, 2], F32)
        g1s = T("g1s", [128, 2], F32)
        be1s = T("be1s", [128, 2], F32)
        g2s = T("g2s", [128, 2], F32)
        be2s = T("be2s", [128, 2], F32)
        stt = T("stt", [128, 2, 2, c.NCH], F32)
        ccp = T("ccp", [128, 4], F32)
        gst = T("gst", [128, 8], F32)
        kdt = T("kdt", [128, 16], F32)
        kd1 = T("kd1", [128, 4], F32)
        kd2 = T("kd2", [128, 4], F32)
        ot = T("ot", [128, 2, 512], F32)

        pa = [P("pa0", [128, HID]), P("pa1", [128, HID])]
        pb = [P("pb0", [128, HID]), P("pb1", [128, HID])]
        pt = [P("pt0", [128, 2, 128], BF), P("pt1", [128, 2, 128], BF)]
        p3 = [P(f"p3{i}", [128, 512]) for i in range(2)]

        s_pre = S("s_pre"); s_ms = S("s_ms")
        s_slab = [S(f"s_slab{i}") for i in range(c.R_XT)]
        s_p1ps = S("s_p1ps"); s_p1ev = S("s_p1ev")
        s_p1st = [S(f"s_p1st{i}") for i in range(2)]
        s_idx = [S(f"s_idx{i}") for i in range(c.R_IXB)]
        s_rv = [S(f"s_rv{i}") for i in range(c.R_RVB)]
        s_g = [S(f"s_g{i}") for i in range(8)]
        s_s = S("s_s")
        s_pg = S("s_pg"); s_e1 = S("s_e1"); s_pt = S("s_pt"); s_hacc = S("s_hacc")
        s_3ps = S("s_3ps"); s_3c = S("s_3c"); s_sq = S("s_sq"); s_h1 = S("s_h1"); s_oev = S("s_oev")
        s_ost = [S(f"s_ost{i}") for i in range(2)]
        s_stf = S("s_stf"); s_cio = S("s_cio"); s_cc = S("s_cc")
        s_kd = S("s_kd")

        N_PRE = 11 * 16
        batch_rgs = [min(4, c.RG - 4 * b) for b in range(c.HB)]
        # phase-3 chunk rows
        rows_t = [min(512, c.RPC - 512 * t) for t in range(c.NCH)]
        rreal_t = [max(0, min(rows_t[t], c.ROWS_REAL - 512 * t)) for t in range(c.NCH)]
        assert all(r > 0 for r in rreal_t)
        NT = 2 * c.NCH          # tiles per phase-3 pass

        def gq(gi):
            return divmod(gi, WPC)  # (quarter, window)

        with nc.Block() as block:

            @block.gpsimd
            def _(g: bass.BassGpSimd):
                g.load_library(mlp)
                # ---- preloads (11 DMAs; casts ride SWDGE) ----
                g.dma_start(wg[:, :, :], _ap(wgcn, 0, [[HID, 128], [128 * HID, 4], [1, HID]])).then_inc(s_pre, 16)
                g.dma_start(w1s[:, :, :, :], _ap(w1, 0, [[HID, 128], [128 * HID, 2], [128, 2], [1, 128]])).then_inc(s_pre, 16)
                g.dma_start(w2s[:, :, :, :], _ap(w2, 0, [[HID, 128], [128 * HID, 2], [128, 2], [1, 128]])).then_inc(s_pre, 16)
                g.dma_start(io_sb[:, :], iota_in[:, :]).then_inc(s_pre, 16)
                g.dma_start(id_sb[:, :], ident_in[:, :]).then_inc(s_pre, 16)
                g.dma_start(bg[:, :], bgcn2[:, :]).then_inc(s_pre, 16)
                g.dma_start(g1s[:, :], g1v[:, :]).then_inc(s_pre, 16)
                g.dma_start(be1s[:, :], be1v[:, :]).then_inc(s_pre, 16)
                g.dma_start(g2s[:, :], g2v[:, :]).then_inc(s_pre, 16)
                g.dma_start(be2s[:, :], be2v[:, :]).then_inc(s_pre, 16)
                g.dma_start(cnt[:, :], gcnt[:, :]).then_inc(s_pre, 16)
                # ---- phase 2: quarter-gated gathers (4 SWDGE queues) ----
                g.wait_ge(s_ms, 8)
                with g.register("cntreg") as creg:
                    for q in range(Q):
                        nb = c.QSL * (q + 1)  # h-store batches needed
                        g.wait_ge(s_p1st[0], 16 * ((nb + 1) // 2))
                        g.wait_ge(s_p1st[1], 16 * (nb // 2))
                        for w in range(WPC):
                            gi = q * WPC + w
                            ib = gi // c.IB
                            g.wait_ge(s_idx[ib % c.R_IXB], 16 * (ib // c.R_IXB + 1))
                            if gi >= 8:
                                g.wait_ge(s_pg, gi - 7)
                            g.reg_load(creg, _ap(cnt, gi, [[NG, 1], [1, 1]]))
                            g.dma_gather(
                                gr[:, gi % 8, :, :],
                                htabs[q][:, :],
                                ixs[:, ib % c.R_IXB, gi % c.IB, :],
                                c.GSLOT, creg, HID, single_packet=False,
                                queue_num=w % 4,
                            ).then_inc(s_g[gi % 8], 16)
                # ---- phase 3: stats AllReduce x2 ----
                g.wait_ge(s_stf, 1)
                g.dma_start(cc1i[:, :], ccp[:, :]).then_inc(s_cio, 16)
                g.wait_ge(s_cio, 16)
                g.collective_compute("AllReduce", OP.add, replica_groups=[list(range(c.CORES))],
                                     ins=[cc1i.ap().opt()], outs=[cc1o.ap().opt()]).then_inc(s_cc, 1)
                g.wait_ge(s_cc, 1)
                g.dma_start(gst[:, 0:4], cc1o[:, :]).then_inc(s_cio, 16)
                g.wait_ge(s_stf, 2)
                g.dma_start(cc2i[:, :], ccp[:, :]).then_inc(s_cio, 16)
                g.wait_ge(s_cio, 48)
                g.collective_compute("AllReduce", OP.add, replica_groups=[list(range(c.CORES))],
                                     ins=[cc2i.ap().opt()], outs=[cc2o.ap().opt()]).then_inc(s_cc, 1)
                g.wait_ge(s_cc, 2)
                g.dma_start(gst[:, 4:8], cc2o[:, :]).then_inc(s_cio, 16)
                g.wait_ge(s_cio, 64)

            @block.sync
            def _(sp):
                # phase-1 slab loads + h-table stores (pure bf16 copies) and
                # idx/rrval streams, statically interleaved.
                def slab(sl):
                    if sl >= c.R_XT:
                        sp.wait_ge(s_p1ps, 4 * (sl - c.R_XT) + 4)
                    sp.dma_start(
                        xts[:, sl % c.R_XT, :, :],
                        _ap(xT, 512 * sl, [[c.TABROWS, 128], [128 * c.TABROWS, 4], [1, 512]]),
                    ).then_inc(s_slab[sl % c.R_XT], 16)

                def store(b):
                    nt = batch_rgs[b]
                    sp.wait_ge(s_p1ev, min(4 * b + nt, c.RG))
                    qb, lb = divmod(b, c.QSL)
                    sp.dma_start(
                        _ap(htabs[qb], lb * 512 * HID, [[HID, 128], [128 * HID, nt], [1, HID]]),
                        ht[:, b % 2, 0:nt, :],
                    ).then_inc(s_p1st[b % 2], 16)

                def idx_stream(ib):
                    if ib >= c.R_IXB:
                        sp.wait_ge(s_pg, c.IB * (ib - c.R_IXB) + c.IB)
                    w8 = c.IB * 8 * CQ
                    sp.dma_start(ixs[:, ib % c.R_IXB, :, :],
                                 idxw[:, ib * w8:(ib + 1) * w8]).then_inc(s_idx[ib % c.R_IXB], 16)

                def rv_stream(rb):
                    if rb >= c.R_RVB:
                        sp.wait_ge(s_s, CQ * c.IB * (rb - c.R_RVB + 1))
                    w2 = c.IB * 2 * CQ
                    sp.dma_start(rvs[:, rb % c.R_RVB, :, :],
                                 rrval[:, rb * w2:(rb + 1) * w2]).then_inc(s_rv[rb % c.R_RVB], 16)

                # section A: quarter-0 slabs/stores + pre-streams
                # interleave: idx and rv alternating so both rings fill early
                pre = [x for pair in zip(
                    [("i", b) for b in range(c.R_IXB)],
                    [("r", b) for b in range(c.R_IXB)]) for x in pair] + \
                    [("r", b) for b in range(c.R_IXB, c.R_RVB)]
                pi = 0
                for sl in range(c.QSL):
                    slab(sl)
                    for _k in range(3):
                        if pi < len(pre):
                            kind, gg = pre[pi]; pi += 1
                            (idx_stream if kind == "i" else rv_stream)(gg)
                    if sl >= 2:
                        store(sl - 2)
                while pi < len(pre):
                    kind, gg = pre[pi]; pi += 1
                    (idx_stream if kind == "i" else rv_stream)(gg)
                for b in range(max(0, c.QSL - 2), c.QSL):
                    store(b)
                # sections q=0..3: streams ahead + next quarter's slabs/stores
                for q in range(Q):
                    base_sl = c.QSL * (q + 1)
                    if q < 3:
                        for s0 in range(min(2, c.QSL)):
                            slab(base_sl + s0)
                    for w in range(WPC):
                        gsl = q * WPC + w
                        if q < 3:
                            if w % 2 == 0 and w // 2 + 2 < c.QSL:
                                slab(base_sl + 2 + w // 2)
                            elif w % 2 == 1 and (w - 1) // 2 < c.QSL:
                                store(base_sl + (w - 1) // 2)
                        # streams emitted 2-3 batches late so their ring-
                        # protect waits trail PE progress with slack
                        if gsl % c.IB == 0 and gsl >= 16:
                            ib = c.R_IXB + (gsl - 16) // c.IB
                            if ib < c.NB:
                                idx_stream(ib)
                        if gsl % c.IB == 4 and gsl >= 28:
                            rb = c.R_RVB + (gsl - 28) // c.IB
                            if rb < c.NB:
                                rv_stream(rb)
                # phase 3 out stores
                for i in range(NT):
                    hf, t = divmod(i, c.NCH)
                    sp.wait_ge(s_oev, i + 1)
                    sp.dma_start(
                        _ap(outT, hf * 128 * c.RPC + t * 512, [[c.RPC, 128], [1, rows_t[t]]]),
                        ot[:, i % 2, 0:rows_t[t]],
                    ).then_inc(s_ost[i % 2], 16)
                sp.wait_ge(s_ost[0], 16 * ((NT + 1) // 2))
                sp.wait_ge(s_ost[1], 16 * (NT // 2))

            @block.tensor
            def _(pe: bass.BassTensorEngine):
                pe.wait_ge(s_pre, N_PRE)

                def rg_mm(rg):
                    sl = rg // 4
                    pe.wait_ge(s_slab[sl % c.R_XT], 16 * (sl // c.R_XT + 1))
                    if rg >= 2:
                        pe.wait_ge(s_p1ev, rg - 1)
                    j = rg % 4
                    for kc in range(4):
                        mm = pe.matmul(
                            pa[rg % 2][:, :],
                            xts[:, (rg // 4) % c.R_XT, kc, 128 * j:128 * (j + 1)],
                            wg[:, kc, :],
                            start=(kc == 0), stop=(kc == 3),
                        )
                        if kc == 3:
                            mm.then_inc(s_p1ps, 1)

                def transpose(v):
                    pe.wait_ge(s_e1, v + 1)
                    if v >= 2:
                        pe.wait_ge(s_hacc, v - 1)
                    for i in range(2):
                        pe.matmul(pt[v % 2][:, i, :], ev1[:, v % 2, 128 * i:128 * (i + 1)],
                                  id_sb[:, :], is_transpose=True, start=True, stop=True).then_inc(s_pt, 1)

                def group_mm(gi):
                    pe.wait_ge(s_g[gi % 8], 16 * (gi // 8 + 1))
                    pe.wait_ge(s_s, CQ * (gi + 1))
                    if gi >= 2:
                        pe.wait_ge(s_e1, gi - 1)
                    for ch in range(CQ):
                        mm = pe.matmul(
                            pb[gi % 2][:, :],
                            ss[:, gi % 6, ch, :],
                            gr[:, gi % 8, ch, :],
                            start=(ch == 0), stop=(ch == CQ - 1),
                        )
                        if ch == CQ - 1:
                            mm.then_inc(s_pg, 1)
                    if gi >= 1:
                        transpose(gi - 1)

                for rg in range(c.QRG):
                    rg_mm(rg)
                for q in range(Q):
                    for w in range(WPC):
                        gi = q * WPC + w
                        group_mm(gi)
                        if q < 3:
                            rg_mm(c.QRG * (q + 1) + 2 * w)
                            rg_mm(c.QRG * (q + 1) + 2 * w + 1)
                transpose(NG - 1)
                # ---- phase 3: 4 passes x (2 halves x NCH chunks) ----
                pe.wait_ge(s_hacc, NG)
                for i in range(4 * NT):
                    p, j = divmod(i, NT)
                    hf, t = divmod(j, c.NCH)
                    if i >= 2:
                        pp, jj = divmod(i - 2, NT)
                        if pp == 0:
                            pe.wait_ge(s_sq, jj + 1)
                        elif pp == 1:
                            pe.wait_ge(s_h1, jj + 1)
                        elif pp == 2:
                            pe.wait_ge(s_sq, NT + jj + 1)
                        else:
                            pe.wait_ge(s_oev, jj + 1)
                    if p == 2:
                        pe.wait_ge(s_h1, c.NCH + t + 1)
                    ws = w1s if p < 2 else w2s
                    src = hat if p < 2 else h1
                    rt = rows_t[t]
                    for ci in range(2):
                        mm = pe.matmul(
                            p3[i % 2][:, 0:rt],
                            ws[:, ci, hf, :],
                            src[:, ci, 512 * t:512 * t + rt],
                            start=(ci == 0), stop=(ci == 1),
                        )
                        if ci == 1:
                            mm.then_inc(s_3ps, 1)

            @block.vector
            def _(v: bass.BassVectorEngine):
                for sl8 in range(8):
                    v.memset(gr[:, sl8, :, :], 0).then_inc(s_ms, 1)
                v.wait_ge(s_pre, N_PRE)
                # ---- phase 2: S-builds + hat accumulation ----
                RVSZ = c.R_RVB * c.IB * 2 * CQ
                io_b = _ap(io_sb, 0, [[128, 128], [0, CQ], [1, 128]])

                def build(gi):
                    rb = gi // c.IB
                    v.wait_ge(s_rv[rb % c.R_RVB], 16 * (rb // c.R_RVB + 1))
                    if gi >= 6:
                        v.wait_ge(s_pg, gi - 5)
                    rbase = ((rb % c.R_RVB) * c.IB + gi % c.IB) * 2 * CQ
                    rr_b = _ap(rvs, rbase, [[RVSZ, 128], [2, CQ], [0, 128]])
                    val_b = _ap(rvs, rbase + 1, [[RVSZ, 128], [2, CQ], [0, 128]])
                    v.scalar_tensor_tensor(
                        ss[:, gi % 6, :, :], io_b, 0.0, rr_b,
                        OP.bypass, OP.is_equal,
                    )
                    v.tensor_mul(ss[:, gi % 6, :, :], ss[:, gi % 6, :, :], val_b).then_inc(s_s, CQ)

                HATSZ = 2 * c.RPC

                def acc(vv):
                    qv, wv = gq(vv)
                    v.wait_ge(s_pt, 2 * (vv + 1))
                    hat_full = _ap(hat, 128 * wv, [[HATSZ, 128], [c.RPC, 2], [1, 128]])
                    pt_full = pt[vv % 2][:, :, :]
                    if qv == 0:
                        v.tensor_scalar(hat_full, pt_full, 0.0, None, OP.add).then_inc(s_hacc, 1)
                    elif qv < 3:
                        v.tensor_add(hat_full, pt_full, hat_full).then_inc(s_hacc, 1)
                    else:
                        for i in range(2):
                            op = v.scalar_tensor_tensor(
                                hat[:, i, 128 * wv:128 * (wv + 1)],
                                pt[vv % 2][:, i, :], bg[:, i:i + 1],
                                hat[:, i, 128 * wv:128 * (wv + 1)],
                                OP.add, OP.add,
                            )
                            if i == 1:
                                op.then_inc(s_hacc, 1)

                for gi in range(NG):
                    build(gi)
                    if gi >= 3:
                        acc(gi - 3)
                for vv in range(max(0, NG - 3), NG):
                    acc(vv)
                # ---- phase 3 ----
                for layer in range(2):
                    base = 0 if layer == 0 else 2 * NT
                    for j in range(NT):
                        hf, t = divmod(j, c.NCH)
                        v.wait_ge(s_3ps, base + j + 1)
                        rr = rreal_t[t]
                        psl = p3[(base + j) % 2]
                        v.tensor_reduce(stt[:, hf, 0, t:t + 1], psl[:, 0:rr],
                                        mybir.AxisListType.X, OP.add).then_inc(s_3c, 1)
                    v.wait_ge(s_sq, NT * (layer + 1))
                    v.drain()
                    v.tensor_reduce(ccp[:, 0:1], stt[:, 0, 0, :], mybir.AxisListType.X, OP.add)
                    v.tensor_reduce(ccp[:, 1:2], stt[:, 0, 1, :], mybir.AxisListType.X, OP.add)
                    v.tensor_reduce(ccp[:, 2:3], stt[:, 1, 0, :], mybir.AxisListType.X, OP.add)
                    v.tensor_reduce(ccp[:, 3:4], stt[:, 1, 1, :], mybir.AxisListType.X, OP.add)
                    v.drain().then_inc(s_stf, 1)
                    # finalize after AllReduce
                    v.wait_ge(s_cio, 32 + 32 * layer)
                    gof = 4 * layer
                    sums = _ap(gst, gof, [[8, 128], [2, 2]])
                    sqs = _ap(gst, gof + 1, [[8, 128], [2, 2]])
                    inv_n = 1.0 / c.N
                    v.tensor_scalar(kdt[:, 0:2], sums, inv_n, None, OP.mult)
                    v.tensor_scalar(kdt[:, 2:4], sqs, inv_n, None, OP.mult)
                    v.drain()
                    v.tensor_mul(kdt[:, 4:6], kdt[:, 0:2], kdt[:, 0:2])
                    v.drain()
                    v.tensor_sub(kdt[:, 6:8], kdt[:, 2:4], kdt[:, 4:6])
                    v.drain()
                    v.tensor_scalar(kdt[:, 6:8], kdt[:, 6:8], c.EPS, None, OP.add)
                    v.drain().then_inc(s_kd, 1)
                    v.wait_ge(s_kd, 2 + 3 * layer)
                    v.reciprocal(kdt[:, 10:12], kdt[:, 8:10])
                    v.drain()
                    kd = kd1 if layer == 0 else kd2
                    gv = g1s if layer == 0 else g2s
                    bev = be1s if layer == 0 else be2s
                    v.tensor_mul(kd[:, 0:2], gv[:, :], kdt[:, 10:12])
                    v.drain()
                    v.tensor_mul(kdt[:, 12:14], kdt[:, 0:2], kd[:, 0:2])
                    v.drain()
                    v.tensor_sub(kd[:, 2:4], bev[:, :], kdt[:, 12:14])
                    v.drain().then_inc(s_kd, 1)

            @block.scalar
            def _(a: bass.BassScalarEngine):
                a.wait_ge(s_pre, N_PRE)

                def rg_ev(rg):
                    a.wait_ge(s_p1ps, rg + 1)
                    b = rg // 4
                    if b >= 2 and rg % 4 == 0:
                        a.wait_ge(s_p1st[b % 2], 16 * ((b - 2) // 2 + 1))
                    a.activation(ht[:, b % 2, rg % 4, :], pa[rg % 2][:, :], AF.Identity).then_inc(s_p1ev, 1)

                def group_ev(gi):
                    a.wait_ge(s_pg, gi + 1)
                    if gi >= 2:
                        a.wait_ge(s_pt, 2 * (gi - 1))
                    a.activation(ev1[:, gi % 2, :], pb[gi % 2][:, :], AF.Identity).then_inc(s_e1, 1)

                for rg in range(c.QRG):
                    rg_ev(rg)
                for q in range(Q):
                    for w in range(WPC):
                        gi = q * WPC + w
                        group_ev(gi)
                        if q < 3:
                            rg_ev(c.QRG * (q + 1) + 2 * w)
                            rg_ev(c.QRG * (q + 1) + 2 * w + 1)
                # ---- phase 3 ----
                for layer in range(2):
                    sbase = 0 if layer == 0 else 2 * NT
                    for j in range(NT):
                        hf, t = divmod(j, c.NCH)
                        a.wait_ge(s_3ps, sbase + j + 1)
                        a.wait_ge(s_3c, NT * layer + j + 1)
                        rr = rreal_t[t]
                        psl = p3[(sbase + j) % 2]
                        a.activation(psl[:, 0:rr], psl[:, 0:rr], AF.Square,
                                     accum_out=stt[:, hf, 1, t:t + 1]).then_inc(s_sq, 1)
                    # sqrt step for k/d
                    a.wait_ge(s_kd, 1 + 3 * layer)
                    a.sqrt(kdt[:, 8:10], kdt[:, 6:8]).then_inc(s_kd, 1)
                    a.wait_ge(s_kd, 3 + 3 * layer)
                    kd = kd1 if layer == 0 else kd2
                    pbase = NT if layer == 0 else 3 * NT
                    for j in range(NT):
                        hf, t = divmod(j, c.NCH)
                        a.wait_ge(s_3ps, pbase + j + 1)
                        rt = rows_t[t]
                        psl = p3[(pbase + j) % 2]
                        if layer == 0:
                            a.activation(h1[:, hf, 512 * t:512 * t + rt], psl[:, 0:rt], AF.Relu,
                                         bias=kd[:, 2 + hf:3 + hf], scale=kd[:, hf:hf + 1]).then_inc(s_h1, 1)
                        else:
                            if j >= 2:
                                a.wait_ge(s_ost[j % 2], 16 * ((j - 2) // 2 + 1))
                            a.activation(ot[:, j % 2, 0:rt], psl[:, 0:rt], AF.Identity,
                                         bias=kd[:, 2 + hf:3 + hf], scale=kd[:, hf:hf + 1]).then_inc(s_oev, 1)

        nc.compile()
    return nc


# ---------------------------------------------------------------------------
# host-side preprocessing
# ---------------------------------------------------------------------------

def preprocess(x, edge_row, edge_col, edge_val, c: Cfg):
    N, E, WPC, Q, SEG = c.N, len(edge_row), c.WPC, c.Q, c.SEG
    deg = np.bincount(edge_row, minlength=N)
    order = np.argsort(-deg, kind="stable")
    rank = np.empty(N, np.int64)
    rank[order] = np.arange(N)
    core = (rank % c.CORES).astype(np.int32)

    win_of = np.empty(N, np.int32)
    slot_of = np.empty(N, np.int32)
    caps = np.full(WPC, 128, np.int32)
    tail = c.ROWS_REAL - 128 * (WPC - 1)
    caps[WPC - 1] = tail if tail > 0 else 128
    assert caps.sum() >= c.ROWS_REAL
    for k in range(c.CORES):
        nodes = order[k::c.CORES]
        heap = [(0, w) for w in range(WPC)]
        heapq.heapify(heap)
        fill = np.zeros(WPC, np.int32)
        for n in nodes:
            s, w = heapq.heappop(heap)
            win_of[n] = w
            slot_of[n] = fill[w]
            fill[w] += 1
            if fill[w] < caps[w]:
                heapq.heappush(heap, (s + int(deg[n]), w))
    ek = core[edge_row]
    ew = win_of[edge_row]
    er = slot_of[edge_row]
    eq = (edge_col // SEG).astype(np.int64)
    erel = (edge_col - eq * SEG).astype(np.int16)
    # q-major bucket key: (core, quarter, window)
    key = ((ek.astype(np.int64) * Q + eq) * WPC + ew)
    sidx = np.argsort(key, kind="stable")
    key_s = key[sidx]
    ngroups = c.CORES * c.NG
    counts = np.bincount(key_s, minlength=ngroups)
    cnt128 = np.maximum((counts + 127) // 128, 1)
    cq_needed = int(cnt128.max())
    if cq_needed > c.CQ:
        return None, cq_needed  # caller rebuilds with larger CQ
    GSLOT = c.GSLOT
    starts = np.zeros(ngroups, np.int64)
    starts[1:] = np.cumsum(counts)[:-1]
    pos = np.arange(E) - starts[key_s]
    gslot = key_s * GSLOT + pos
    TOT = ngroups * GSLOT
    idx_sl = np.full(TOT, -1, np.int16)
    rr_sl = np.zeros(TOT, np.float32)
    val_sl = np.zeros(TOT, np.float32)
    idx_sl[gslot] = erel[sidx]
    rr_sl[gslot] = er[sidx].astype(np.float32)
    val_sl[gslot] = np.asarray(edge_val, np.float32)[sidx]
    sig = np.arange(TOT, dtype=np.int64) % GSLOT
    gof = np.arange(TOT, dtype=np.int64) // GSLOT
    padmask = (sig >= counts[gof]) & (sig < cnt128[gof] * 128)
    idx_sl[padmask] = 0
    gcnt_all = (cnt128 * 128).astype(np.int32)

    xTp = np.zeros((c.IN_C, c.TABROWS), dtype=BF16)
    xTp[:, :N] = np.asarray(x, np.float32).T.astype(BF16)

    per_core = []
    idx_c = idx_sl.reshape(c.CORES, c.NG, GSLOT)
    rr_c = rr_sl.reshape(c.CORES, c.NG, c.CQ, 128)
    val_c = val_sl.reshape(c.CORES, c.NG, c.CQ, 128)
    for k in range(c.CORES):
        # per-group 16-wrap: idx j -> (partition j%16, col g*8CQ + j//16)
        a = idx_c[k].reshape(c.NG, 8 * c.CQ, 16)
        w16 = np.transpose(a, (2, 0, 1)).reshape(16, c.NG * 8 * c.CQ)
        idxw_k = np.ascontiguousarray(np.tile(w16, (8, 1)))
        # per-group rr/val: [128, NG, CQ, 2] -> [128, NG*2CQ]
        rrT = np.transpose(rr_c[k], (2, 0, 1))   # [128, NG, CQ]
        valT = np.transpose(val_c[k], (2, 0, 1))
        rrval_k = np.ascontiguousarray(
            np.stack([rrT, valT], axis=-1).reshape(128, -1)).astype(BF16)
        per_core.append(dict(idxw=idxw_k, rrval=rrval_k,
                             gcnt=np.ascontiguousarray(
                                 np.tile(gcnt_all.reshape(c.CORES, -1)[k:k + 1], (128, 1)))))
    meta = dict(core=core, win_of=win_of, slot_of=slot_of, xTp=xTp)
    return (per_core, meta), None


def make_in_maps(inputs, c: Cfg):
    res, cq_needed = preprocess(inputs["x"], np.asarray(inputs["edge_row"]),
                                np.asarray(inputs["edge_col"]), np.asarray(inputs["edge_val"]), c)
    if res is None:
        return None, cq_needed
    per_core, meta = res
    iota = np.broadcast_to(np.arange(128, dtype=np.float32), (128, 128)).astype(BF16)
    ident = np.eye(128, dtype=np.float32).astype(BF16)

    def v2(b):
        return np.ascontiguousarray(np.asarray(b, np.float32).reshape(2, 128).T)

    shared = dict(
        xT=meta["xTp"],
        wgcn=np.asarray(inputs["W_gcn"], np.float32),
        w1=np.asarray(inputs["W1"], np.float32),
        w2=np.asarray(inputs["W2"], np.float32),
        bgcn2=v2(inputs["b_gcn"]), g1v=v2(inputs["g1"]), be1v=v2(inputs["be1"]),
        g2v=v2(inputs["g2"]), be2v=v2(inputs["be2"]),
        iota=np.ascontiguousarray(iota), ident=np.ascontiguousarray(ident),
    )
    in_maps = [dict(shared, **pc) for pc in per_core]
    return (in_maps, meta), None


def unshard(results, meta, c: Cfg):
    core, win_of, slot_of = meta["core"], meta["win_of"], meta["slot_of"]
    out = np.empty((c.N, c.OUT_C), np.float32)
    rowpos = win_of.astype(np.int64) * 128 + slot_of
    for k in range(c.CORES):
        o = np.asarray(results[k]["outT"]).reshape(2, 128, c.RPC)
        o = np.transpose(o, (2, 0, 1)).reshape(c.RPC, c.OUT_C)
        nodes_k = np.flatnonzero(core == k)
        out[nodes_k] = o[rowpos[nodes_k]]
    return out


_NC_CACHE = {}


def get_nc(c: Cfg):
    key = (c.N, c.SEG, c.WPC, c.CQ, c.PH)
    if key not in _NC_CACHE:
        _NC_CACHE[key] = build(c)
    return _NC_CACHE[key]


def kernel(**inputs):
    c = Cfg()
    while True:
        res, cq_needed = make_in_maps(inputs, c)
        if res is not None:
            break
        c = Cfg(CQ=cq_needed)
    in_maps, meta = res
    nc = get_nc(c)
    r = run_bass_kernel_spmd(nc, in_maps, list(range(c.CORES)))
    return unshard(r.results, meta, c)
